# revision 1
# baseline (speedup 1.0000x reference)
import sys

sys.path.insert(0, "/opt/trn_rl_repo")

import numpy as np

D_MODEL = 1024
NUM_HEADS = 16
HEAD_DIM = 64
B = 2
S = 2048
N_CORES = 8
HG = 4          # head-groups (cores per batch)
HPC = 4         # heads per core
DL = 256        # local feature width per core (HPC * HEAD_DIM)

_cache = {}
last_exec_time_ns = None


def _build(has_qkvb):
    import concourse.bacc as bacc
    import concourse.mybir as mybir
    import concourse.tile as tile

    F32 = mybir.dt.float32
    F32R = mybir.dt.float32r
    Exp = mybir.ActivationFunctionType.Exp
    mult = mybir.AluOpType.mult
    is_ge = mybir.AluOpType.is_ge

    nc = bacc.Bacc("TRN2", target_bir_lowering=False, debug=False)
    xT_d = nc.dram_tensor("xT", (D_MODEL, S), F32, kind="ExternalInput")
    wq_d = nc.dram_tensor("wqkvT", (D_MODEL, 3 * DL), F32, kind="ExternalInput")
    wo_d = nc.dram_tensor("woT", (DL, D_MODEL), F32, kind="ExternalInput")
    if has_qkvb:
        qb_d = nc.dram_tensor("qb", (1, 3 * DL), F32, kind="ExternalInput")
    out_d = nc.dram_tensor("out", (S, D_MODEL), F32, kind="ExternalOutput")

    def r(ap):
        return ap.bitcast(F32R)

    with tile.TileContext(nc) as tc:
        with tc.tile_pool(name="persist", bufs=1) as persist:
            # Q/K packed per head-pair p: partitions 0:64 head 2p, 64:128 head 2p+1
            QT = [persist.tile([128, S], F32, name=f"QT{p}") for p in range(2)]
            KT = [persist.tile([128, S], F32, name=f"KT{p}") for p in range(2)]
            # V augmented: per s-tile block of 128 cols: [V dims 64 | ones 64]
            Vaug = [persist.tile([128, S], F32, name=f"Vg{h}") for h in range(HPC)]
            # prebaked causal band masks: mask[t][k, q] = 1 if q >= k + 128t else 0
            maskt = [persist.tile([128, 512], F32, name=f"mask{t}") for t in range(4)]
            for t in range(4):
                nc.vector.memset(maskt[t][:], 1.0)
                w = 128 * (t + 1)
                nc.gpsimd.affine_select(
                    out=r(maskt[t][:, 0:w]), in_=r(maskt[t][:, 0:w]),
                    pattern=[[1, w]],
                    channel_multiplier=-1,
                    base=-128 * t,
                    compare_op=is_ge,
                    fill=0.0,
                )

            with tc.tile_pool(name="work", bufs=1) as work:
                with tc.tile_pool(name="projin", bufs=1) as projin, \
                     tc.tile_pool(name="pproj", bufs=1, space="PSUM") as pproj:
                    xt = [projin.tile([128, S], F32, name=f"xt{i}") for i in range(8)]
                    wq = [projin.tile([128, 3 * DL], F32, name=f"wq{i}") for i in range(8)]
                    for i in range(8):
                        nc.sync.dma_start(out=r(xt[i][:]), in_=r(xT_d[128 * i:128 * (i + 1), :]))
                        nc.sync.dma_start(out=r(wq[i][:]), in_=r(wq_d[128 * i:128 * (i + 1), :]))
                    if has_qkvb:
                        qb_t = projin.tile([1, 3 * DL], F32, name="qb_t")
                        nc.sync.dma_start(out=r(qb_t[:]), in_=r(qb_d[:]))
                        ones_t = projin.tile([1, 512], F32, name="ones_t")
                        nc.vector.memset(ones_t[:], 1.0)

                    # ---- QK projection: mi 0/1 -> QT[0/1], 2/3 -> KT[0/1]
                    for mi in range(4):
                        dst = QT[mi] if mi < 2 else KT[mi - 2]
                        for n in range(4):
                            psq = pproj.tile([128, 512], F32, tag="qk", bufs=2, name="psq")
                            for i in range(8):
                                nc.tensor.matmul(
                                    out=psq[:],
                                    lhsT=r(wq[i][:, 128 * mi:128 * (mi + 1)]),
                                    rhs=r(xt[i][:, 512 * n:512 * (n + 1)]),
                                    start=(i == 0),
                                    stop=(i == 7 and not has_qkvb),
                                )
                            if has_qkvb:
                                nc.tensor.matmul(
                                    out=psq[:],
                                    lhsT=r(qb_t[0:1, 128 * mi:128 * (mi + 1)]),
                                    rhs=r(ones_t[0:1, :]),
                                    start=False, stop=True,
                                )
                            nc.vector.tensor_copy(out=r(dst[:, 512 * n:512 * (n + 1)]), in_=psq[:])

                    # ---- V projection into Vaug (interleaved [V|ones] blocks)
                    for h in range(HPC):
                        nc.vector.memset(Vaug[h][:], 1.0)
                    for st in range(16):
                        psv = pproj.tile([128, DL], F32, tag="v", bufs=2, name="psv")
                        for i in range(8):
                            nc.tensor.matmul(
                                out=psv[:],
                                lhsT=r(xt[i][:, 128 * st:128 * (st + 1)]),
                                rhs=r(wq[i][:, 512:768]),
                                start=(i == 0),
                                stop=(i == 7 and not has_qkvb),
                            )
                        if has_qkvb:
                            nc.tensor.matmul(
                                out=psv[:],
                                lhsT=r(ones_t[0:1, 0:128]),
                                rhs=r(qb_t[0:1, 512:768]),
                                start=False, stop=True,
                            )
                        for h in range(HPC):
                            nc.vector.tensor_copy(
                                out=r(Vaug[h][:, 128 * st:128 * st + 64]),
                                in_=psv[:, 64 * h:64 * h + 64],
                            )

                with tc.tile_pool(name="persist2", bufs=1) as persist2:
                    # ctx pair-packed: head 2p at partitions 0:64, head 2p+1 at 64:128
                    ctxp = [persist2.tile([128, S], F32, name=f"ctxp{p}") for p in range(2)]
                    wop = [persist2.tile([128, D_MODEL], F32, name=f"wop{p}") for p in range(2)]
                    for p in range(2):
                        nc.sync.dma_start(out=r(wop[p][:]), in_=r(wo_d[128 * p:128 * (p + 1), :]))

                    # ---- attention
                    with tc.tile_pool(name="pattn", bufs=1, space="PSUM") as pattn:
                        def issue_scores(p, j, m):
                            psS0 = pattn.tile([128, 512], F32, tag="s0", bufs=2, name="psS0")
                            psS1 = pattn.tile([128, 512], F32, tag="s1", bufs=2, name="psS1")
                            nc.tensor.matmul(
                                out=psS0[:],
                                lhsT=r(KT[p][0:64, 128 * m:128 * (m + 1)]),
                                rhs=r(QT[p][0:64, 512 * j:512 * (j + 1)]),
                                start=True, stop=True,
                            )
                            nc.tensor.matmul(
                                out=psS1[:],
                                lhsT=r(KT[p][64:128, 128 * m:128 * (m + 1)]),
                                rhs=r(QT[p][64:128, 512 * j:512 * (j + 1)]),
                                start=True, stop=True,
                            )
                            return psS0, psS1

                        pjs = [(p, j) for p in range(2) for j in range(4)]
                        pending = issue_scores(*pjs[0], 0)
                        for pi, (p, j) in enumerate(pjs):
                            mlast = 4 * j + 3
                            psA = pattn.tile([128, 512], F32, tag="a", bufs=2, name="psA")
                            psB = pattn.tile([128, 512], F32, tag="b", bufs=2, name="psB")
                            for m in range(4 * j + 4):
                                psS0, psS1 = pending
                                if m < mlast:
                                    pending = issue_scores(p, j, m + 1)
                                elif pi + 1 < len(pjs):
                                    pending = issue_scores(*pjs[pi + 1], 0)
                                e0 = work.tile([128, 512], F32, tag="e0", bufs=3, name="e0")
                                e1 = work.tile([128, 512], F32, tag="e1", bufs=3, name="e1")
                                t = m - 4 * j
                                if t >= 0:
                                    # band tile: cols < 128t are fully below the causal
                                    # diagonal -> zero; exp only live cols, mask only the
                                    # 128-col partial band
                                    w0 = 128 * t
                                    nc.scalar.activation(
                                        r(e0[:, w0:512]), psS0[:, w0:512], Exp, scale=0.125)
                                    nc.scalar.activation(
                                        r(e1[:, w0:512]), psS1[:, w0:512], Exp, scale=0.125)
                                    nc.vector.tensor_tensor(
                                        out=r(e0[:, w0:w0 + 128]), in0=e0[:, w0:w0 + 128],
                                        in1=maskt[t][:, w0:w0 + 128], op=mult)
                                    nc.vector.tensor_tensor(
                                        out=r(e1[:, w0:w0 + 128]), in0=e1[:, w0:w0 + 128],
                                        in1=maskt[t][:, w0:w0 + 128], op=mult)
                                else:
                                    nc.scalar.activation(r(e0[:]), psS0[:], Exp, scale=0.125)
                                    nc.scalar.activation(r(e1[:]), psS1[:], Exp, scale=0.125)
                                lo = 128 * t if t > 0 else 0
                                nc.tensor.matmul(
                                    out=psA[:, lo:512],
                                    lhsT=r(Vaug[2 * p][:, 128 * m:128 * (m + 1)]),
                                    rhs=r(e0[:, lo:512]),
                                    start=(m == 0), stop=(m == mlast),
                                )
                                nc.tensor.matmul(
                                    out=psB[:, lo:512],
                                    lhsT=r(Vaug[2 * p + 1][:, 128 * m:128 * (m + 1)]),
                                    rhs=r(e1[:, lo:512]),
                                    start=(m == 0), stop=(m == mlast),
                                )
                            # normalize: ctxp[p][0:64|64:128, j] = psX[0:64] / sums
                            sums = work.tile([64, 512], F32, tag="sums", bufs=2, name="sums")
                            nc.vector.tensor_copy(out=sums[:], in_=psA[64:128, :])
                            rec = work.tile([64, 512], F32, tag="rec", bufs=2, name="rec")
                            nc.vector.reciprocal_approx_fast(rec[:], sums[:])
                            nc.vector.tensor_tensor(
                                out=r(ctxp[p][0:64, 512 * j:512 * (j + 1)]),
                                in0=psA[0:64, :],
                                in1=rec[:],
                                op=mult,
                            )
                            sums2 = work.tile([64, 512], F32, tag="sums", bufs=2, name="sums")
                            nc.vector.tensor_copy(out=sums2[:], in_=psB[64:128, :])
                            rec2 = work.tile([64, 512], F32, tag="rec", bufs=2, name="rec")
                            nc.vector.reciprocal_approx_fast(rec2[:], sums2[:])
                            codd = work.tile([64, 512], F32, tag="codd", bufs=2, name="codd")
                            nc.vector.tensor_tensor(
                                out=codd[:], in0=psB[0:64, :], in1=rec2[:], op=mult)
                            nc.vector.tensor_copy(
                                out=r(ctxp[p][64:128, 512 * j:512 * (j + 1)]), in_=codd[:])

                    # ---- output projection
                    with tc.tile_pool(name="outst", bufs=1) as outst, \
                         tc.tile_pool(name="pout", bufs=1, space="PSUM") as pout:
                        for qm in range(16):
                            stage = outst.tile([128, D_MODEL], F32, tag="st", bufs=3, name="stage")
                            for n in range(2):
                                pso = pout.tile([128, 512], F32, tag=f"o{n}", bufs=2, name="pso")
                                for p in range(2):
                                    nc.tensor.matmul(
                                        out=pso[:],
                                        lhsT=r(ctxp[p][:, 128 * qm:128 * (qm + 1)]),
                                        rhs=r(wop[p][:, 512 * n:512 * (n + 1)]),
                                        start=(p == 0), stop=(p == 1),
                                    )
                                nc.vector.tensor_copy(out=stage[:, 512 * n:512 * (n + 1)], in_=pso[:])
                            nc.sync.dma_start(out=out_d[128 * qm:128 * (qm + 1), :], in_=stage[:])

    nc.finalize()
    return nc


def kernel(x, qkv_w, qkv_b, out_w, out_b):
    from concourse import bass_utils
    global last_exec_time_ns

    x = np.ascontiguousarray(np.asarray(x, dtype=np.float32))
    qkv_w = np.asarray(qkv_w, dtype=np.float32)
    qkv_b = np.asarray(qkv_b, dtype=np.float32)
    out_w = np.asarray(out_w, dtype=np.float32)
    out_b = np.asarray(out_b, dtype=np.float32)

    has_qkvb = bool(np.any(qkv_b))
    if has_qkvb not in _cache:
        _cache[has_qkvb] = _build(has_qkvb)
    nc = _cache[has_qkvb]

    in_maps = []
    for c in range(N_CORES):
        b, hg = divmod(c, HG)
        xT = np.ascontiguousarray(x[b].T)
        rows = np.concatenate([
            qkv_w[DL * hg:DL * (hg + 1)],
            qkv_w[D_MODEL + DL * hg:D_MODEL + DL * (hg + 1)],
            qkv_w[2 * D_MODEL + DL * hg:2 * D_MODEL + DL * (hg + 1)],
        ], axis=0)
        wqkvT = np.ascontiguousarray(rows.T)
        woT = np.ascontiguousarray(out_w[:, DL * hg:DL * (hg + 1)].T)
        m = {"xT": xT, "wqkvT": wqkvT, "woT": woT}
        if has_qkvb:
            m["qb"] = np.concatenate([
                qkv_b[DL * hg:DL * (hg + 1)],
                qkv_b[D_MODEL + DL * hg:D_MODEL + DL * (hg + 1)],
                qkv_b[2 * D_MODEL + DL * hg:2 * D_MODEL + DL * (hg + 1)],
            ]).reshape(1, 3 * DL).astype(np.float32)
        in_maps.append(m)

    res = bass_utils.run_bass_kernel_spmd(nc, in_maps, core_ids=list(range(N_CORES)))
    last_exec_time_ns = res.exec_time_ns

    out = np.zeros((B, S, D_MODEL), dtype=np.float32)
    for c in range(N_CORES):
        b, hg = divmod(c, HG)
        out[b] += res.results[c]["out"]
    out += out_b[None, None, :]
    return out



# revision 2
# speedup vs baseline: 1.8262x; 1.8262x over previous
import sys

sys.path.insert(0, "/opt/trn_rl_repo")

import numpy as np

D_MODEL = 1024
NUM_HEADS = 16
HEAD_DIM = 64
B = 2
S = 2048
N_CORES = 8
HG = 4          # head-groups (cores per batch)
HPC = 4         # heads per core
DL = 256        # local feature width per core (HPC * HEAD_DIM)

_cache = {}
last_exec_time_ns = None


def _build(has_qkvb):
    import concourse.bacc as bacc
    import concourse.mybir as mybir
    import concourse.tile as tile

    F32 = mybir.dt.float32
    BF16 = mybir.dt.bfloat16
    Exp = mybir.ActivationFunctionType.Exp
    mult = mybir.AluOpType.mult
    is_ge = mybir.AluOpType.is_ge

    nc = bacc.Bacc("TRN2", target_bir_lowering=False, debug=False)
    xT_d = nc.dram_tensor("xT", (D_MODEL, S), BF16, kind="ExternalInput")
    wq_d = nc.dram_tensor("wqkvT", (D_MODEL, 3 * DL), BF16, kind="ExternalInput")
    wo_d = nc.dram_tensor("woT", (DL, D_MODEL), BF16, kind="ExternalInput")
    if has_qkvb:
        qb_d = nc.dram_tensor("qb", (1, 3 * DL), BF16, kind="ExternalInput")
    out_d = nc.dram_tensor("out", (S, D_MODEL), F32, kind="ExternalOutput")

    with tile.TileContext(nc) as tc:
        with tc.tile_pool(name="persist", bufs=1) as persist:
            xt = [persist.tile([128, S], BF16, name=f"xt{i}") for i in range(8)]
            wq = [persist.tile([128, 3 * DL], BF16, name=f"wq{i}") for i in range(8)]
            # Q/K packed per head-pair p: partitions 0:64 head 2p, 64:128 head 2p+1
            QT = [persist.tile([128, S], BF16, name=f"QT{p}") for p in range(2)]
            KT = [persist.tile([128, S], BF16, name=f"KT{p}") for p in range(2)]
            # V augmented, single tile: [pair, st, head-parity, (v|ones), 64]
            VA = persist.tile([128, 2, 16, 2, 2, 64], BF16, name="VA")
            # ctx pair-packed: head 2p at partitions 0:64, head 2p+1 at 64:128
            ctxp = [persist.tile([128, S], BF16, name=f"ctxp{p}") for p in range(2)]
            wop = [persist.tile([128, D_MODEL], BF16, name=f"wop{p}") for p in range(2)]

            # input DMAs: weights first, then x halves
            for i in range(8):
                nc.sync.dma_start(out=wq[i][:], in_=wq_d[128 * i:128 * (i + 1), :])
            for h in range(2):
                for i in range(8):
                    nc.sync.dma_start(
                        out=xt[i][:, 1024 * h:1024 * (h + 1)],
                        in_=xT_d[128 * i:128 * (i + 1), 1024 * h:1024 * (h + 1)],
                    )
            for p in range(2):
                nc.sync.dma_start(out=wop[p][:], in_=wo_d[128 * p:128 * (p + 1), :])

            nc.vector.memset(VA[:], 1.0)

            with tc.tile_pool(name="work", bufs=1) as work, \
                 tc.tile_pool(name="psum", bufs=1, space="PSUM") as psum:

                if has_qkvb:
                    qb_t = persist.tile([1, 3 * DL], BF16, name="qb_t")
                    nc.sync.dma_start(out=qb_t[:], in_=qb_d[:])
                    ones_t = persist.tile([1, 512], BF16, name="ones_t")
                    nc.vector.memset(ones_t[:], 1.0)

                # ACT exp-table preload during DMA wait
                warm = work.tile([1, 16], F32, name="warm")
                nc.vector.memset(warm[:], 0.0)
                nc.scalar.activation(warm[:], warm[:], Exp, scale=1.0)

                def qk_proj(mi, n):
                    # psq [128 qk-dims, 512 keys] for pair mi (0,1=Q pairs; 2,3=K pairs)
                    dst = QT[mi] if mi < 2 else KT[mi - 2]
                    psq = psum.tile([128, 512], F32, tag="p", bufs=2, name="psq")
                    for i in range(8):
                        nc.tensor.matmul(
                            out=psq[:],
                            lhsT=wq[i][:, 128 * mi:128 * (mi + 1)],
                            rhs=xt[i][:, 512 * n:512 * (n + 1)],
                            start=(i == 0),
                            stop=(i == 7 and not has_qkvb),
                        )
                    if has_qkvb:
                        nc.tensor.matmul(
                            out=psq[:],
                            lhsT=qb_t[0:1, 128 * mi:128 * (mi + 1)],
                            rhs=ones_t[0:1, :],
                            start=False, stop=True,
                        )
                    nc.vector.tensor_copy(out=dst[:, 512 * n:512 * (n + 1)], in_=psq[:])

                def v_proj(st):
                    # psv [128 keys, (pair, parity, 64)]
                    psv = psum.tile([128, 2, 2, 64], F32, tag="p", bufs=2, name="psv")
                    for i in range(8):
                        nc.tensor.matmul(
                            out=psv[:],
                            lhsT=xt[i][:, 128 * st:128 * (st + 1)],
                            rhs=wq[i][:, 512:768],
                            start=(i == 0),
                            stop=(i == 7 and not has_qkvb),
                        )
                    if has_qkvb:
                        nc.tensor.matmul(
                            out=psv[:],
                            lhsT=ones_t[0:1, 0:128],
                            rhs=qb_t[0:1, 512:768],
                            start=False, stop=True,
                        )
                    nc.vector.tensor_copy(out=VA[:, :, st, :, 0, :], in_=psv[:])

                def attn_block(p, j):
                    mlast = 4 * j + 3
                    psA = psum.tile([128, 512], F32, tag="a", bufs=1, name="psA")
                    psB = psum.tile([128, 512], F32, tag="b", bufs=1, name="psB")
                    for m in range(4 * j + 4):
                        t = m - 4 * j
                        w0 = 128 * t if t > 0 else 0
                        psS = psum.tile([128, 2, 512], F32, tag="s", bufs=2, name="psS")
                        # concurrent row-tiled score pair: head 2p rows 0:64,
                        # head 2p+1 rows 64:128
                        nc.tensor.matmul(
                            out=psS[:, 0, w0:512],
                            lhsT=KT[p][0:64, 128 * m:128 * (m + 1)],
                            rhs=QT[p][0:64, 512 * j + w0:512 * (j + 1)],
                            start=True, stop=True,
                            tile_position=(0, 0),
                        )
                        nc.tensor.matmul(
                            out=psS[:, 1, w0:512],
                            lhsT=KT[p][64:128, 128 * m:128 * (m + 1)],
                            rhs=QT[p][64:128, 512 * j + w0:512 * (j + 1)],
                            start=True, stop=True,
                            tile_position=(64, 0),
                        )
                        e = work.tile([128, 2, 512], BF16, tag="e", bufs=3, name="e")
                        nc.scalar.activation(
                            e[:, :, w0:512], psS[:, :, w0:512], Exp, scale=0.125)
                        if t >= 0:
                            # zero the upper-triangular part of the 128-col
                            # diagonal band for both heads in one op:
                            # keep where col - key >= 0
                            nc.gpsimd.affine_select(
                                out=e[:, :, w0:w0 + 128],
                                in_=e[:, :, w0:w0 + 128],
                                pattern=[[0, 2], [1, 128]],
                                channel_multiplier=-1,
                                base=0,
                                compare_op=is_ge,
                                fill=0.0,
                            )
                        lo = w0
                        nc.tensor.matmul(
                            out=psA[:, lo:512],
                            lhsT=VA[:, p, m, 0, :, :],
                            rhs=e[:, 0, lo:512],
                            start=(m == 0), stop=(m == mlast),
                        )
                        nc.tensor.matmul(
                            out=psB[:, lo:512],
                            lhsT=VA[:, p, m, 1, :, :],
                            rhs=e[:, 1, lo:512],
                            start=(m == 0), stop=(m == mlast),
                        )
                    # normalize: ctxp[p][0:64|64:128, j-block] = psX[0:64] / sums
                    sums = work.tile([64, 512], F32, tag="sums", bufs=2, name="sums")
                    nc.vector.tensor_copy(out=sums[:], in_=psA[64:128, :])
                    rec = work.tile([64, 512], F32, tag="rec", bufs=2, name="rec")
                    nc.vector.reciprocal_approx_fast(rec[:], sums[:])
                    nc.vector.tensor_tensor(
                        out=ctxp[p][0:64, 512 * j:512 * (j + 1)],
                        in0=psA[0:64, :],
                        in1=rec[:],
                        op=mult,
                    )
                    sums2 = work.tile([64, 512], F32, tag="sums", bufs=2, name="sums")
                    nc.vector.tensor_copy(out=sums2[:], in_=psB[64:128, :])
                    rec2 = work.tile([64, 512], F32, tag="rec", bufs=2, name="rec")
                    nc.vector.reciprocal_approx_fast(rec2[:], sums2[:])
                    codd = work.tile([64, 512], BF16, tag="codd", bufs=2, name="codd")
                    nc.vector.tensor_tensor(
                        out=codd[:], in0=psB[0:64, :], in1=rec2[:], op=mult)
                    nc.vector.tensor_copy(
                        out=ctxp[p][64:128, 512 * j:512 * (j + 1)], in_=codd[:])

                def out_proj(qm):
                    stage = work.tile([128, D_MODEL], F32, tag="st", bufs=2, name="stage")
                    for nh in range(2):
                        pso = psum.tile([128, 512], F32, tag="p", bufs=2, name="pso")
                        for p in range(2):
                            nc.tensor.matmul(
                                out=pso[:],
                                lhsT=ctxp[p][:, 128 * qm:128 * (qm + 1)],
                                rhs=wop[p][:, 512 * nh:512 * (nh + 1)],
                                start=(p == 0), stop=(p == 1),
                            )
                        nc.vector.tensor_copy(out=stage[:, 512 * nh:512 * (nh + 1)], in_=pso[:])
                    nc.sync.dma_start(out=out_d[128 * qm:128 * (qm + 1), :], in_=stage[:])

                # fused schedule: per j-block emit the minimal projection
                # prerequisites, then the attention blocks, then the output
                # projection rows they unlock.  The priority-heap scheduler
                # fills attention-phase PE gaps with later proj/outproj work.
                for j in range(4):
                    qk_proj(0, j)
                    qk_proj(2, j)
                    for st in range(4 * j, 4 * j + 4):
                        v_proj(st)
                    attn_block(0, j)
                    qk_proj(1, j)
                    qk_proj(3, j)
                    attn_block(1, j)
                    for qm in range(4 * j, 4 * j + 4):
                        out_proj(qm)

    nc.finalize()
    return nc


def kernel(x, qkv_w, qkv_b, out_w, out_b):
    from concourse import bass_utils
    import ml_dtypes
    global last_exec_time_ns

    BF = ml_dtypes.bfloat16

    x = np.asarray(x, dtype=np.float32)
    qkv_w = np.asarray(qkv_w, dtype=np.float32)
    qkv_b = np.asarray(qkv_b, dtype=np.float32)
    out_w = np.asarray(out_w, dtype=np.float32)
    out_b = np.asarray(out_b, dtype=np.float32)

    has_qkvb = bool(np.any(qkv_b))
    if has_qkvb not in _cache:
        _cache[has_qkvb] = _build(has_qkvb)
    nc = _cache[has_qkvb]

    in_maps = []
    for c in range(N_CORES):
        b, hg = divmod(c, HG)
        xT = np.ascontiguousarray(x[b].T.astype(BF))
        rows = np.concatenate([
            qkv_w[DL * hg:DL * (hg + 1)],
            qkv_w[D_MODEL + DL * hg:D_MODEL + DL * (hg + 1)],
            qkv_w[2 * D_MODEL + DL * hg:2 * D_MODEL + DL * (hg + 1)],
        ], axis=0)
        wqkvT = np.ascontiguousarray(rows.T.astype(BF))
        woT = np.ascontiguousarray(out_w[:, DL * hg:DL * (hg + 1)].T.astype(BF))
        m = {"xT": xT, "wqkvT": wqkvT, "woT": woT}
        if has_qkvb:
            m["qb"] = np.concatenate([
                qkv_b[DL * hg:DL * (hg + 1)],
                qkv_b[D_MODEL + DL * hg:D_MODEL + DL * (hg + 1)],
                qkv_b[2 * D_MODEL + DL * hg:2 * D_MODEL + DL * (hg + 1)],
            ]).reshape(1, 3 * DL).astype(BF)
        in_maps.append(m)

    res = bass_utils.run_bass_kernel_spmd(nc, in_maps, core_ids=list(range(N_CORES)))
    last_exec_time_ns = res.exec_time_ns

    out = np.zeros((B, S, D_MODEL), dtype=np.float32)
    for c in range(N_CORES):
        b, hg = divmod(c, HG)
        out[b] += np.asarray(res.results[c]["out"], dtype=np.float32)
    out += out_b[None, None, :]
    return out


# revision 10
# speedup vs baseline: 2.1857x; 1.1968x over previous
import sys

sys.path.insert(0, "/opt/trn_rl_repo")

import numpy as np

D_MODEL = 1024
NUM_HEADS = 16
HEAD_DIM = 64
B = 2
S = 2048
N_CORES = 8
HG = 4          # head-groups (cores per batch)
HPC = 4         # heads per core
DL = 256        # local feature width per core (HPC * HEAD_DIM)

_cache = {}
last_exec_time_ns = None


def _build(has_qkvb):
    import concourse.bacc as bacc
    import concourse.mybir as mybir
    import concourse.tile as tile

    F32 = mybir.dt.float32
    BF16 = mybir.dt.bfloat16
    Exp = mybir.ActivationFunctionType.Exp
    mult = mybir.AluOpType.mult
    is_ge = mybir.AluOpType.is_ge

    nc = bacc.Bacc("TRN2", target_bir_lowering=False, debug=False)
    xT_d = nc.dram_tensor("xT", (D_MODEL, S), BF16, kind="ExternalInput")
    wq_d = nc.dram_tensor("wqkvT", (D_MODEL, 3 * DL), BF16, kind="ExternalInput")
    wo_d = nc.dram_tensor("woT", (DL, D_MODEL), BF16, kind="ExternalInput")
    if has_qkvb:
        qb_d = nc.dram_tensor("qb", (1, 3 * DL), BF16, kind="ExternalInput")
    out_d = nc.dram_tensor("out", (S, D_MODEL), F32, kind="ExternalOutput")

    with tile.TileContext(nc) as tc:
        with tc.tile_pool(name="persist", bufs=1) as persist:
            xt = [persist.tile([128, S], BF16, name=f"xt{i}") for i in range(8)]
            wq = [persist.tile([128, 3 * DL], BF16, name=f"wq{i}") for i in range(8)]
            # Q/K packed per head-pair p: partitions 0:64 head 2p, 64:128 head 2p+1
            QT = [persist.tile([128, S], BF16, name=f"QT{p}") for p in range(2)]
            KT = [persist.tile([128, S], BF16, name=f"KT{p}") for p in range(2)]
            # V augmented, single tile: [pair, st, head-parity, (v|ones), 64]
            VA = persist.tile([128, 2, 16, 2, 2, 64], BF16, name="VA")
            # ctx pair-packed: head 2p at partitions 0:64, head 2p+1 at 64:128
            ctxp = [persist.tile([128, S], BF16, name=f"ctxp{p}") for p in range(2)]
            wop = [persist.tile([128, D_MODEL], BF16, name=f"wop{p}") for p in range(2)]

            # input DMAs spread across engine queues for issue parallelism
            for i in range(4):
                nc.sync.dma_start(out=wq[i][:], in_=wq_d[128 * i:128 * (i + 1), :])
                nc.sync.dma_start(
                    out=xt[i][:, 0:1024], in_=xT_d[128 * i:128 * (i + 1), 0:1024])
            for i in range(4, 8):
                nc.gpsimd.dma_start(out=wq[i][:], in_=wq_d[128 * i:128 * (i + 1), :])
                nc.gpsimd.dma_start(
                    out=xt[i][:, 0:1024], in_=xT_d[128 * i:128 * (i + 1), 0:1024])
            for i in range(4):
                nc.sync.dma_start(
                    out=xt[i][:, 1024:2048],
                    in_=xT_d[128 * i:128 * (i + 1), 1024:2048])
            for i in range(4, 8):
                nc.gpsimd.dma_start(
                    out=xt[i][:, 1024:2048],
                    in_=xT_d[128 * i:128 * (i + 1), 1024:2048])
            for p in range(2):
                nc.scalar.dma_start(out=wop[p][:], in_=wo_d[128 * p:128 * (p + 1), :])

            # ones columns of VA (v columns are overwritten by v_proj copies)
            nc.vector.memset(VA[:], 1.0)

            with tc.tile_pool(name="work", bufs=1) as work, \
                 tc.tile_pool(name="psum", bufs=1, space="PSUM") as psum:

                if has_qkvb:
                    qb_t = persist.tile([1, 3 * DL], BF16, name="qb_t")
                    nc.sync.dma_start(out=qb_t[:], in_=qb_d[:])
                    ones_t = persist.tile([1, 512], BF16, name="ones_t")
                    nc.vector.memset(ones_t[:], 1.0)

                # ACT exp-table preload during DMA wait
                warm = work.tile([1, 16], F32, name="warm")
                nc.vector.memset(warm[:], 0.0)
                nc.scalar.activation(warm[:], warm[:], Exp, scale=1.0)

                # ---- filler emitters (each yields per-matmul granularity) ----

                def qk_proj(mi, n):
                    # psq [128 qk-dims, 512 keys]; mi 0,1 = Q pairs; 2,3 = K pairs
                    dst = QT[mi] if mi < 2 else KT[mi - 2]
                    psq = psum.tile([128, 512], F32, tag="p", bufs=2, name="psq")
                    for i in range(8):
                        yield
                        nc.tensor.matmul(
                            out=psq[:],
                            lhsT=wq[i][:, 128 * mi:128 * (mi + 1)],
                            rhs=xt[i][:, 512 * n:512 * (n + 1)],
                            start=(i == 0),
                            stop=(i == 7 and not has_qkvb),
                        )
                    if has_qkvb:
                        nc.tensor.matmul(
                            out=psq[:],
                            lhsT=qb_t[0:1, 128 * mi:128 * (mi + 1)],
                            rhs=ones_t[0:1, :],
                            start=False, stop=True,
                        )
                    nc.vector.tensor_copy(out=dst[:, 512 * n:512 * (n + 1)], in_=psq[:])

                def v_proj(st):
                    # psv [128 keys, (pair, parity, 64)]
                    psv = psum.tile([128, 2, 2, 64], F32, tag="p", bufs=2, name="psv")
                    for i in range(8):
                        yield
                        nc.tensor.matmul(
                            out=psv[:],
                            lhsT=xt[i][:, 128 * st:128 * (st + 1)],
                            rhs=wq[i][:, 512:768],
                            start=(i == 0),
                            stop=(i == 7 and not has_qkvb),
                        )
                    if has_qkvb:
                        nc.tensor.matmul(
                            out=psv[:],
                            lhsT=ones_t[0:1, 0:128],
                            rhs=qb_t[0:1, 512:768],
                            start=False, stop=True,
                        )
                    nc.vector.tensor_copy(out=VA[:, :, st, :, 0, :], in_=psv[:])

                def out_proj(qm):
                    stage = work.tile([128, D_MODEL], F32, tag="st", bufs=2, name="stage")
                    for nh in range(2):
                        pso = psum.tile([128, 512], F32, tag="p", bufs=2, name="pso")
                        for p in range(2):
                            yield
                            nc.tensor.matmul(
                                out=pso[:],
                                lhsT=ctxp[p][:, 128 * qm:128 * (qm + 1)],
                                rhs=wop[p][:, 512 * nh:512 * (nh + 1)],
                                start=(p == 0), stop=(p == 1),
                            )
                        nc.vector.tensor_copy(
                            out=stage[:, 512 * nh:512 * (nh + 1)], in_=pso[:])
                    nc.sync.dma_start(out=out_d[128 * qm:128 * (qm + 1), :], in_=stage[:])

                # global attention step sequence and filler queue with gates.
                # gate = number of attention steps that must be EMITTED before
                # the filler unit may start (keeps FIFO order consistent with
                # data deps, e.g. out_proj needs its ctx block normalized).
                steps = [(p, j, m) for j in range(4) for p in range(2)
                         for m in range(4 * j + 4)]
                n_steps = len(steps)   # 80
                # step index right after block (p,j) finishes:
                end_of = {}
                acc = 0
                for j in range(4):
                    for p in range(2):
                        acc += 4 * j + 4
                        end_of[(p, j)] = acc

                filler = []   # list of (gate, key, generator)

                def add(gate, key, gen):
                    filler.append((gate, key, gen))

                # outproj of j-block may only be emitted AFTER normalize(1,j)
                # has been emitted, which happens while processing the step at
                # index end_of[(1,j)] — so its gate is end_of+1.
                add(0, ("qk", 0, 0), qk_proj(0, 0))
                add(0, ("qk", 2, 0), qk_proj(2, 0))
                for st in range(0, 4):
                    add(0, ("v", st), v_proj(st))
                add(0, ("qk", 1, 0), qk_proj(1, 0))
                add(0, ("qk", 3, 0), qk_proj(3, 0))
                add(0, ("qk", 0, 1), qk_proj(0, 1))
                add(0, ("qk", 2, 1), qk_proj(2, 1))
                for st in range(4, 8):
                    add(0, ("v", st), v_proj(st))
                add(0, ("qk", 1, 1), qk_proj(1, 1))
                add(0, ("qk", 3, 1), qk_proj(3, 1))
                for qm in range(0, 4):
                    add(end_of[(1, 0)] + 1, ("op", qm), out_proj(qm))
                add(0, ("qk", 0, 2), qk_proj(0, 2))
                add(0, ("qk", 2, 2), qk_proj(2, 2))
                for st in range(8, 12):
                    add(0, ("v", st), v_proj(st))
                add(0, ("qk", 1, 2), qk_proj(1, 2))
                add(0, ("qk", 3, 2), qk_proj(3, 2))
                for qm in range(4, 8):
                    add(end_of[(1, 1)] + 1, ("op", qm), out_proj(qm))
                add(0, ("qk", 0, 3), qk_proj(0, 3))
                add(0, ("qk", 2, 3), qk_proj(2, 3))
                for st in range(12, 16):
                    add(0, ("v", st), v_proj(st))
                add(0, ("qk", 1, 3), qk_proj(1, 3))
                add(0, ("qk", 3, 3), qk_proj(3, 3))
                for qm in range(8, 12):
                    add(end_of[(1, 2)] + 1, ("op", qm), out_proj(qm))
                for qm in range(12, 16):
                    add(end_of[(1, 3)] + 1, ("op", qm), out_proj(qm))

                total_filler_mms = 8 * (16 + 16) + 4 * 16   # qk+v groups, outproj
                fill_state = {"emitted": 0, "idx": 0}
                produced = set()

                def drain_filler(step_idx, budget):
                    done = 0
                    while done < budget and fill_state["idx"] < len(filler):
                        gate, key, gen = filler[fill_state["idx"]]
                        if gate > step_idx:
                            break
                        try:
                            next(gen)
                            done += 1
                            fill_state["emitted"] += 1
                        except StopIteration:
                            produced.add(key)
                            fill_state["idx"] += 1
                    return done

                def require(step_idx, *keys):
                    # force-drain filler (in order, respecting gates) until
                    # the named units have fully emitted
                    while any(k not in produced for k in keys):
                        if drain_filler(step_idx, 1) == 0:
                            raise RuntimeError(f"unsatisfiable requires {keys}")

                def scores_exp(p, j, m):
                    t = m - 4 * j
                    w0 = 128 * t if t > 0 else 0
                    psS = psum.tile([128, 2, 512], F32, tag="s", bufs=2, name="psS")
                    nc.tensor.matmul(
                        out=psS[:, 0, w0:512],
                        lhsT=KT[p][0:64, 128 * m:128 * (m + 1)],
                        rhs=QT[p][0:64, 512 * j + w0:512 * (j + 1)],
                        start=True, stop=True,
                        tile_position=(0, 0),
                    )
                    nc.tensor.matmul(
                        out=psS[:, 1, w0:512],
                        lhsT=KT[p][64:128, 128 * m:128 * (m + 1)],
                        rhs=QT[p][64:128, 512 * j + w0:512 * (j + 1)],
                        start=True, stop=True,
                        tile_position=(64, 0),
                    )
                    e = work.tile([128, 2, 512], BF16, tag="e", bufs=3, name="e")
                    nc.scalar.activation(
                        e[:, :, w0:512], psS[:, :, w0:512], Exp, scale=0.125)
                    if t >= 0:
                        # causal band: keep where col - key >= 0 (both heads)
                        nc.gpsimd.affine_select(
                            out=e[:, :, w0:w0 + 128],
                            in_=e[:, :, w0:w0 + 128],
                            pattern=[[0, 2], [1, 128]],
                            channel_multiplier=-1,
                            base=0,
                            compare_op=is_ge,
                            fill=0.0,
                        )
                    return e, w0

                def av(acc, p, j, m, e, lo):
                    psA, psB = acc
                    mlast = 4 * j + 3
                    nc.tensor.matmul(
                        out=psA[:, lo:512],
                        lhsT=VA[:, p, m, 0, :, :],
                        rhs=e[:, 0, lo:512],
                        start=(m == 0), stop=(m == mlast),
                    )
                    nc.tensor.matmul(
                        out=psB[:, lo:512],
                        lhsT=VA[:, p, m, 1, :, :],
                        rhs=e[:, 1, lo:512],
                        start=(m == 0), stop=(m == mlast),
                    )

                def normalize(acc, p, j):
                    psA, psB = acc
                    sums = work.tile([64, 512], F32, tag="sums", bufs=2, name="sums")
                    nc.vector.tensor_copy(out=sums[:], in_=psA[64:128, :])
                    rec = work.tile([64, 512], F32, tag="rec", bufs=2, name="rec")
                    nc.vector.reciprocal_approx_fast(rec[:], sums[:])
                    nc.vector.tensor_tensor(
                        out=ctxp[p][0:64, 512 * j:512 * (j + 1)],
                        in0=psA[0:64, :],
                        in1=rec[:],
                        op=mult,
                    )
                    sums2 = work.tile([64, 512], F32, tag="sums", bufs=2, name="sums")
                    nc.vector.tensor_copy(out=sums2[:], in_=psB[64:128, :])
                    rec2 = work.tile([64, 512], F32, tag="rec", bufs=2, name="rec")
                    nc.vector.reciprocal_approx_fast(rec2[:], sums2[:])
                    codd = work.tile([64, 512], BF16, tag="codd", bufs=2, name="codd")
                    nc.vector.tensor_tensor(
                        out=codd[:], in0=psB[0:64, :], in1=rec2[:], op=mult)
                    nc.vector.tensor_copy(
                        out=ctxp[p][64:128, 512 * j:512 * (j + 1)], in_=codd[:])

                # software-pipelined main loop: AV(k-1) is emitted after
                # scores(k) so the PE never head-blocks on exp(k-1); filler
                # (proj / outproj) matmuls pace in to keep the PE dense.
                cur_acc = None
                pend = None   # (acc, p, j, m, e, lo)
                for idx, (p, j, m) in enumerate(steps):
                    if m == 0:
                        # new block: fresh accumulators (WAR on previous
                        # block's normalize is absorbed by boundary filler)
                        cur_acc = (
                            psum.tile([128, 512], F32, tag="a", bufs=1, name="psA"),
                            psum.tile([128, 512], F32, tag="b", bufs=1, name="psB"),
                        )
                        drain_filler(idx, 4)
                        # Q pair of this block and K pair cols up to 512(j+1)
                        # must be fully emitted before its scores
                        require(idx, ("qk", p, j), ("qk", 2 + p, j))
                    e, w0 = scores_exp(p, j, m)
                    rem_steps = n_steps - idx
                    rem = (total_filler_mms - fill_state["emitted"])
                    budget = max(2, (rem + rem_steps - 1) // rem_steps)
                    drain_filler(idx, max(1, budget // 2))
                    if pend is not None:
                        pacc, pp, pj, pm, pe, plo = pend
                        require(idx, ("v", pm))
                        av(pacc, pp, pj, pm, pe, plo)
                        if pm == 4 * pj + 3:
                            normalize(pacc, pp, pj)
                    pend = (cur_acc, p, j, m, e, w0)
                    drain_filler(idx, budget - budget // 2)
                pacc, pp, pj, pm, pe, plo = pend
                require(n_steps, ("v", pm))
                av(pacc, pp, pj, pm, pe, plo)
                normalize(pacc, pp, pj)
                # drain any remaining filler (final outproj blocks)
                while fill_state["idx"] < len(filler):
                    if drain_filler(n_steps + 1, 1 << 30) == 0:
                        break

    nc.finalize()
    return nc


def kernel(x, qkv_w, qkv_b, out_w, out_b):
    from concourse import bass_utils
    import ml_dtypes
    global last_exec_time_ns

    BF = ml_dtypes.bfloat16

    x = np.asarray(x, dtype=np.float32)
    qkv_w = np.asarray(qkv_w, dtype=np.float32)
    qkv_b = np.asarray(qkv_b, dtype=np.float32)
    out_w = np.asarray(out_w, dtype=np.float32)
    out_b = np.asarray(out_b, dtype=np.float32)

    has_qkvb = bool(np.any(qkv_b))
    if has_qkvb not in _cache:
        _cache[has_qkvb] = _build(has_qkvb)
    nc = _cache[has_qkvb]

    in_maps = []
    for c in range(N_CORES):
        b, hg = divmod(c, HG)
        xT = np.ascontiguousarray(x[b].T.astype(BF))
        rows = np.concatenate([
            qkv_w[DL * hg:DL * (hg + 1)],
            qkv_w[D_MODEL + DL * hg:D_MODEL + DL * (hg + 1)],
            qkv_w[2 * D_MODEL + DL * hg:2 * D_MODEL + DL * (hg + 1)],
        ], axis=0)
        wqkvT = np.ascontiguousarray(rows.T.astype(BF))
        woT = np.ascontiguousarray(out_w[:, DL * hg:DL * (hg + 1)].T.astype(BF))
        m = {"xT": xT, "wqkvT": wqkvT, "woT": woT}
        if has_qkvb:
            m["qb"] = np.concatenate([
                qkv_b[DL * hg:DL * (hg + 1)],
                qkv_b[D_MODEL + DL * hg:D_MODEL + DL * (hg + 1)],
                qkv_b[2 * D_MODEL + DL * hg:2 * D_MODEL + DL * (hg + 1)],
            ]).reshape(1, 3 * DL).astype(BF)
        in_maps.append(m)

    res = bass_utils.run_bass_kernel_spmd(nc, in_maps, core_ids=list(range(N_CORES)))
    last_exec_time_ns = res.exec_time_ns

    out = np.zeros((B, S, D_MODEL), dtype=np.float32)
    for c in range(N_CORES):
        b, hg = divmod(c, HG)
        out[b] += np.asarray(res.results[c]["out"], dtype=np.float32)
    out += out_b[None, None, :]
    return out


# revision 11
# speedup vs baseline: 2.2659x; 1.0367x over previous
import sys

sys.path.insert(0, "/opt/trn_rl_repo")

import numpy as np

D_MODEL = 1024
NUM_HEADS = 16
HEAD_DIM = 64
B = 2
S = 2048
N_CORES = 8
HG = 4          # head-groups (cores per batch)
HPC = 4         # heads per core
DL = 256        # local feature width per core (HPC * HEAD_DIM)

_cache = {}
last_exec_time_ns = None


def _build(has_qkvb):
    import concourse.bacc as bacc
    import concourse.mybir as mybir
    import concourse.tile as tile

    F32 = mybir.dt.float32
    BF16 = mybir.dt.bfloat16
    Exp = mybir.ActivationFunctionType.Exp
    mult = mybir.AluOpType.mult
    is_ge = mybir.AluOpType.is_ge

    nc = bacc.Bacc("TRN2", target_bir_lowering=False, debug=False)
    xT_d = nc.dram_tensor("xT", (D_MODEL, S), BF16, kind="ExternalInput")
    wq_d = nc.dram_tensor("wqkvT", (D_MODEL, 3 * DL), BF16, kind="ExternalInput")
    wo_d = nc.dram_tensor("woT", (DL, D_MODEL), BF16, kind="ExternalInput")
    if has_qkvb:
        qb_d = nc.dram_tensor("qb", (1, 3 * DL), BF16, kind="ExternalInput")
    out_d = nc.dram_tensor("out", (S, D_MODEL), F32, kind="ExternalOutput")

    with tile.TileContext(nc) as tc:
        with tc.tile_pool(name="persist", bufs=1) as persist:
            xt = [persist.tile([128, S], BF16, name=f"xt{i}") for i in range(8)]
            wq = [persist.tile([128, 3 * DL], BF16, name=f"wq{i}") for i in range(8)]
            # Q/K packed per head-pair p: partitions 0:64 head 2p, 64:128 head 2p+1
            QT = [persist.tile([128, S], BF16, name=f"QT{p}") for p in range(2)]
            KT = [persist.tile([128, S], BF16, name=f"KT{p}") for p in range(2)]
            # V augmented, single tile: [pair, st, head-parity, (v|ones), 64]
            VA = persist.tile([128, 2, 16, 2, 2, 64], BF16, name="VA")
            # ctx pair-packed: head 2p at partitions 0:64, head 2p+1 at 64:128
            ctxp = [persist.tile([128, S], BF16, name=f"ctxp{p}") for p in range(2)]
            wop = [persist.tile([128, D_MODEL], BF16, name=f"wop{p}") for p in range(2)]

            # input DMAs spread across engine queues for issue parallelism
            # first 512 cols of x land first so the n=0 projection group is
            # not serialized behind the full x transfer
            for i in range(4):
                nc.sync.dma_start(out=wq[i][:], in_=wq_d[128 * i:128 * (i + 1), :])
                nc.sync.dma_start(
                    out=xt[i][:, 0:512], in_=xT_d[128 * i:128 * (i + 1), 0:512])
            for i in range(4, 8):
                nc.gpsimd.dma_start(out=wq[i][:], in_=wq_d[128 * i:128 * (i + 1), :])
                nc.gpsimd.dma_start(
                    out=xt[i][:, 0:512], in_=xT_d[128 * i:128 * (i + 1), 0:512])
            for i in range(4):
                nc.sync.dma_start(
                    out=xt[i][:, 512:1024], in_=xT_d[128 * i:128 * (i + 1), 512:1024])
            for i in range(4, 8):
                nc.gpsimd.dma_start(
                    out=xt[i][:, 512:1024], in_=xT_d[128 * i:128 * (i + 1), 512:1024])
            for i in range(4):
                nc.sync.dma_start(
                    out=xt[i][:, 1024:2048],
                    in_=xT_d[128 * i:128 * (i + 1), 1024:2048])
            for i in range(4, 8):
                nc.gpsimd.dma_start(
                    out=xt[i][:, 1024:2048],
                    in_=xT_d[128 * i:128 * (i + 1), 1024:2048])
            for p in range(2):
                nc.scalar.dma_start(out=wop[p][:], in_=wo_d[128 * p:128 * (p + 1), :])

            # ones columns of VA (v columns are overwritten by v_proj copies)
            nc.vector.memset(VA[:], 1.0)

            with tc.tile_pool(name="work", bufs=1) as work, \
                 tc.tile_pool(name="psum", bufs=1, space="PSUM") as psum:

                if has_qkvb:
                    qb_t = persist.tile([1, 3 * DL], BF16, name="qb_t")
                    nc.sync.dma_start(out=qb_t[:], in_=qb_d[:])
                    ones_t = persist.tile([1, 512], BF16, name="ones_t")
                    nc.vector.memset(ones_t[:], 1.0)

                # ACT exp-table preload during DMA wait
                warm = work.tile([1, 16], F32, name="warm")
                nc.vector.memset(warm[:], 0.0)
                nc.scalar.activation(warm[:], warm[:], Exp, scale=1.0)

                # ---- filler emitters (each yields per-matmul granularity) ----

                def qk_proj(mi, n):
                    # psq [128 qk-dims, 512 keys]; mi 0,1 = Q pairs; 2,3 = K pairs
                    dst = QT[mi] if mi < 2 else KT[mi - 2]
                    psq = psum.tile([128, 512], F32, tag="p", bufs=2, name="psq")
                    for i in range(8):
                        yield
                        nc.tensor.matmul(
                            out=psq[:],
                            lhsT=wq[i][:, 128 * mi:128 * (mi + 1)],
                            rhs=xt[i][:, 512 * n:512 * (n + 1)],
                            start=(i == 0),
                            stop=(i == 7 and not has_qkvb),
                        )
                    if has_qkvb:
                        nc.tensor.matmul(
                            out=psq[:],
                            lhsT=qb_t[0:1, 128 * mi:128 * (mi + 1)],
                            rhs=ones_t[0:1, :],
                            start=False, stop=True,
                        )
                    nc.vector.tensor_copy(out=dst[:, 512 * n:512 * (n + 1)], in_=psq[:])

                def v_proj(st):
                    # psv [128 keys, (pair, parity, 64)]
                    psv = psum.tile([128, 2, 2, 64], F32, tag="p", bufs=2, name="psv")
                    for i in range(8):
                        yield
                        nc.tensor.matmul(
                            out=psv[:],
                            lhsT=xt[i][:, 128 * st:128 * (st + 1)],
                            rhs=wq[i][:, 512:768],
                            start=(i == 0),
                            stop=(i == 7 and not has_qkvb),
                        )
                    if has_qkvb:
                        nc.tensor.matmul(
                            out=psv[:],
                            lhsT=ones_t[0:1, 0:128],
                            rhs=qb_t[0:1, 512:768],
                            start=False, stop=True,
                        )
                    nc.vector.tensor_copy(out=VA[:, :, st, :, 0, :], in_=psv[:])

                def out_proj(qm):
                    stage = work.tile([128, D_MODEL], F32, tag="st", bufs=2, name="stage")
                    for nh in range(2):
                        pso = psum.tile([128, 512], F32, tag="p", bufs=2, name="pso")
                        for p in range(2):
                            yield
                            nc.tensor.matmul(
                                out=pso[:],
                                lhsT=ctxp[p][:, 128 * qm:128 * (qm + 1)],
                                rhs=wop[p][:, 512 * nh:512 * (nh + 1)],
                                start=(p == 0), stop=(p == 1),
                            )
                        nc.vector.tensor_copy(
                            out=stage[:, 512 * nh:512 * (nh + 1)], in_=pso[:])
                    nc.sync.dma_start(out=out_d[128 * qm:128 * (qm + 1), :], in_=stage[:])

                # global attention step sequence and filler queue with gates.
                # gate = number of attention steps that must be EMITTED before
                # the filler unit may start (keeps FIFO order consistent with
                # data deps, e.g. out_proj needs its ctx block normalized).
                steps = [(p, j, m) for j in range(4) for p in range(2)
                         for m in range(4 * j + 4)]
                n_steps = len(steps)   # 80
                # step index right after block (p,j) finishes:
                end_of = {}
                acc = 0
                for j in range(4):
                    for p in range(2):
                        acc += 4 * j + 4
                        end_of[(p, j)] = acc

                filler = []   # list of (gate, key, generator)

                def add(gate, key, gen):
                    filler.append((gate, key, gen))

                # outproj of j-block may only be emitted AFTER normalize(1,j)
                # has been emitted, which happens while processing the step at
                # index end_of[(1,j)] — so its gate is end_of+1.
                add(0, ("qk", 0, 0), qk_proj(0, 0))
                add(0, ("qk", 2, 0), qk_proj(2, 0))
                for st in range(0, 4):
                    add(0, ("v", st), v_proj(st))
                add(0, ("qk", 1, 0), qk_proj(1, 0))
                add(0, ("qk", 3, 0), qk_proj(3, 0))
                add(0, ("qk", 0, 1), qk_proj(0, 1))
                add(0, ("qk", 2, 1), qk_proj(2, 1))
                for st in range(4, 8):
                    add(0, ("v", st), v_proj(st))
                add(0, ("qk", 1, 1), qk_proj(1, 1))
                add(0, ("qk", 3, 1), qk_proj(3, 1))
                for qm in range(0, 4):
                    add(end_of[(1, 0)] + 1, ("op", qm), out_proj(qm))
                add(0, ("qk", 0, 2), qk_proj(0, 2))
                add(0, ("qk", 2, 2), qk_proj(2, 2))
                for st in range(8, 12):
                    add(0, ("v", st), v_proj(st))
                add(0, ("qk", 1, 2), qk_proj(1, 2))
                add(0, ("qk", 3, 2), qk_proj(3, 2))
                for qm in range(4, 8):
                    add(end_of[(1, 1)] + 1, ("op", qm), out_proj(qm))
                add(0, ("qk", 0, 3), qk_proj(0, 3))
                add(0, ("qk", 2, 3), qk_proj(2, 3))
                for st in range(12, 16):
                    add(0, ("v", st), v_proj(st))
                add(0, ("qk", 1, 3), qk_proj(1, 3))
                add(0, ("qk", 3, 3), qk_proj(3, 3))
                for qm in range(8, 12):
                    add(end_of[(1, 2)] + 1, ("op", qm), out_proj(qm))
                for qm in range(12, 16):
                    add(end_of[(1, 3)] + 1, ("op", qm), out_proj(qm))

                total_filler_mms = 8 * (16 + 16) + 4 * 16   # qk+v groups, outproj
                fill_state = {"emitted": 0, "idx": 0}
                produced = set()

                def drain_filler(step_idx, budget):
                    done = 0
                    while done < budget and fill_state["idx"] < len(filler):
                        gate, key, gen = filler[fill_state["idx"]]
                        if gate > step_idx:
                            break
                        try:
                            next(gen)
                            done += 1
                            fill_state["emitted"] += 1
                        except StopIteration:
                            produced.add(key)
                            fill_state["idx"] += 1
                    return done

                def require(step_idx, *keys):
                    # force-drain filler (in order, respecting gates) until
                    # the named units have fully emitted
                    while any(k not in produced for k in keys):
                        if drain_filler(step_idx, 1) == 0:
                            raise RuntimeError(f"unsatisfiable requires {keys}")

                def scores_exp(p, j, m):
                    t = m - 4 * j
                    w0 = 128 * t if t > 0 else 0
                    psS = psum.tile([128, 2, 512], F32, tag="s", bufs=2, name="psS")
                    nc.tensor.matmul(
                        out=psS[:, 0, w0:512],
                        lhsT=KT[p][0:64, 128 * m:128 * (m + 1)],
                        rhs=QT[p][0:64, 512 * j + w0:512 * (j + 1)],
                        start=True, stop=True,
                        tile_position=(0, 0),
                    )
                    nc.tensor.matmul(
                        out=psS[:, 1, w0:512],
                        lhsT=KT[p][64:128, 128 * m:128 * (m + 1)],
                        rhs=QT[p][64:128, 512 * j + w0:512 * (j + 1)],
                        start=True, stop=True,
                        tile_position=(64, 0),
                    )
                    e = work.tile([128, 2, 512], BF16, tag="e", bufs=3, name="e")
                    nc.scalar.activation(
                        e[:, :, w0:512], psS[:, :, w0:512], Exp, scale=0.125)
                    if t >= 0:
                        # causal band: keep where col - key >= 0 (both heads)
                        nc.gpsimd.affine_select(
                            out=e[:, :, w0:w0 + 128],
                            in_=e[:, :, w0:w0 + 128],
                            pattern=[[0, 2], [1, 128]],
                            channel_multiplier=-1,
                            base=0,
                            compare_op=is_ge,
                            fill=0.0,
                        )
                    return e, w0

                def av(acc, p, j, m, e, lo):
                    psA, psB = acc
                    mlast = 4 * j + 3
                    nc.tensor.matmul(
                        out=psA[:, lo:512],
                        lhsT=VA[:, p, m, 0, :, :],
                        rhs=e[:, 0, lo:512],
                        start=(m == 0), stop=(m == mlast),
                    )
                    nc.tensor.matmul(
                        out=psB[:, lo:512],
                        lhsT=VA[:, p, m, 1, :, :],
                        rhs=e[:, 1, lo:512],
                        start=(m == 0), stop=(m == mlast),
                    )

                def normalize(acc, p, j):
                    psA, psB = acc
                    sums = work.tile([64, 512], F32, tag="sums", bufs=2, name="sums")
                    nc.vector.tensor_copy(out=sums[:], in_=psA[64:128, :])
                    rec = work.tile([64, 512], F32, tag="rec", bufs=2, name="rec")
                    nc.vector.reciprocal_approx_fast(rec[:], sums[:])
                    nc.vector.tensor_tensor(
                        out=ctxp[p][0:64, 512 * j:512 * (j + 1)],
                        in0=psA[0:64, :],
                        in1=rec[:],
                        op=mult,
                    )
                    sums2 = work.tile([64, 512], F32, tag="sums", bufs=2, name="sums")
                    nc.vector.tensor_copy(out=sums2[:], in_=psB[64:128, :])
                    rec2 = work.tile([64, 512], F32, tag="rec", bufs=2, name="rec")
                    nc.vector.reciprocal_approx_fast(rec2[:], sums2[:])
                    codd = work.tile([64, 512], BF16, tag="codd", bufs=2, name="codd")
                    nc.vector.tensor_tensor(
                        out=codd[:], in0=psB[0:64, :], in1=rec2[:], op=mult)
                    nc.vector.tensor_copy(
                        out=ctxp[p][64:128, 512 * j:512 * (j + 1)], in_=codd[:])

                # software-pipelined main loop: AV(k-1) is emitted after
                # scores(k) so the PE never head-blocks on exp(k-1); filler
                # (proj / outproj) matmuls pace in to keep the PE dense.
                cur_acc = None
                pend = None   # (acc, p, j, m, e, lo)
                for idx, (p, j, m) in enumerate(steps):
                    if m == 0:
                        # new block: fresh accumulators (WAR on previous
                        # block's normalize is absorbed by boundary filler)
                        cur_acc = (
                            psum.tile([128, 512], F32, tag="a", bufs=1, name="psA"),
                            psum.tile([128, 512], F32, tag="b", bufs=1, name="psB"),
                        )
                        drain_filler(idx, 4)
                        # Q pair of this block and K pair cols up to 512(j+1)
                        # must be fully emitted before its scores
                        require(idx, ("qk", p, j), ("qk", 2 + p, j))
                    e, w0 = scores_exp(p, j, m)
                    rem_steps = n_steps - idx
                    rem = (total_filler_mms - fill_state["emitted"])
                    budget = max(2, (rem + rem_steps - 1) // rem_steps)
                    drain_filler(idx, max(1, budget // 2))
                    if pend is not None:
                        pacc, pp, pj, pm, pe, plo = pend
                        require(idx, ("v", pm))
                        av(pacc, pp, pj, pm, pe, plo)
                        if pm == 4 * pj + 3:
                            normalize(pacc, pp, pj)
                    pend = (cur_acc, p, j, m, e, w0)
                    drain_filler(idx, budget - budget // 2)
                pacc, pp, pj, pm, pe, plo = pend
                require(n_steps, ("v", pm))
                av(pacc, pp, pj, pm, pe, plo)
                normalize(pacc, pp, pj)
                # drain any remaining filler (final outproj blocks)
                while fill_state["idx"] < len(filler):
                    if drain_filler(n_steps + 1, 1 << 30) == 0:
                        break

    nc.finalize()
    return nc


def kernel(x, qkv_w, qkv_b, out_w, out_b):
    from concourse import bass_utils
    import ml_dtypes
    global last_exec_time_ns

    BF = ml_dtypes.bfloat16

    x = np.asarray(x, dtype=np.float32)
    qkv_w = np.asarray(qkv_w, dtype=np.float32)
    qkv_b = np.asarray(qkv_b, dtype=np.float32)
    out_w = np.asarray(out_w, dtype=np.float32)
    out_b = np.asarray(out_b, dtype=np.float32)

    has_qkvb = bool(np.any(qkv_b))
    if has_qkvb not in _cache:
        _cache[has_qkvb] = _build(has_qkvb)
    nc = _cache[has_qkvb]

    in_maps = []
    for c in range(N_CORES):
        b, hg = divmod(c, HG)
        xT = np.ascontiguousarray(x[b].T.astype(BF))
        rows = np.concatenate([
            qkv_w[DL * hg:DL * (hg + 1)],
            qkv_w[D_MODEL + DL * hg:D_MODEL + DL * (hg + 1)],
            qkv_w[2 * D_MODEL + DL * hg:2 * D_MODEL + DL * (hg + 1)],
        ], axis=0)
        wqkvT = np.ascontiguousarray(rows.T.astype(BF))
        woT = np.ascontiguousarray(out_w[:, DL * hg:DL * (hg + 1)].T.astype(BF))
        m = {"xT": xT, "wqkvT": wqkvT, "woT": woT}
        if has_qkvb:
            m["qb"] = np.concatenate([
                qkv_b[DL * hg:DL * (hg + 1)],
                qkv_b[D_MODEL + DL * hg:D_MODEL + DL * (hg + 1)],
                qkv_b[2 * D_MODEL + DL * hg:2 * D_MODEL + DL * (hg + 1)],
            ]).reshape(1, 3 * DL).astype(BF)
        in_maps.append(m)

    res = bass_utils.run_bass_kernel_spmd(nc, in_maps, core_ids=list(range(N_CORES)))
    last_exec_time_ns = res.exec_time_ns

    out = np.zeros((B, S, D_MODEL), dtype=np.float32)
    for c in range(N_CORES):
        b, hg = divmod(c, HG)
        out[b] += np.asarray(res.results[c]["out"], dtype=np.float32)
    out += out_b[None, None, :]
    return out
